# revision 16
# baseline (speedup 1.0000x reference)
"""Distributed Trainium2 kernel for the 21-qubit staircase variational circuit.

Math: the circuit is (RY encoding + Rot layer + CNOT chain) x 3 + <Z_w>.
Each CNOT chain is a computational-basis permutation (prefix-XOR), so the
state just before the FINAL chain decomposes exactly, per 8-way shard on
wires 0..2 (most-significant), as a rank-4 sum of outer products
    psi^{(d)}[p, f] = sum_{t<4} U_t[d, p] * W_t[f]
with U_t complex [8,128] (wires 3..9) and W_t complex [2048] (wires 10..20).
The final chain folds into prefix-parity observables
    <Z_w>_final = sum_b |psi[b]|^2 * (-1)^(b_0^...^b_w).

|psi|^2 expands exactly as a rank-16 REAL outer-product sum (Hermitian pair
expansion of the rank-4 amplitude): probs[p,f] = Q^T @ P with Q [16,128] per
shard and P [16,2048] shared; the SA (p-side) sign table then folds into Q
on the host: A'[w,k] = sum_p SA[p,w] Q[k,p] (host work stays O(2^11)).

Each NeuronCore computes its shard's signed-probability field
    po[w, f] = sum_k A'[k,w] P[k,f]   ([21, 2048], K=16 bf16 matmuls into
PSUM at 1 cycle/row) and contracts the f axis against the SF prefix-parity
sign table with DVE scalar_tensor_tensor accumulate. The 2048 f-columns are
split into 6 chunks packed 3-per-PSUM-bank at partition offsets 0/32/64
(the PE's allowed bf16 dst quadrants; f32r only allows offset 0), so the
whole SF contraction is just TWO DVE ops (416/267 cols — bank A is wider
because its DMA half lands first, letting stt_b ride gaplessly behind
stt_a). DMA descriptor generation overlaps across the SP HWDGE slots and
the Pool SWDGE queue; completion semaphores are merged (sf increments the
matmul sem, memzeros the qp sem) so each consumer pays one wait-decode;
the result DMA's completion is covered by the block-exit drain (no trailing
semaphore round-trip). Operands: qp bf16 (rel err ~1e-3 vs the 2e-2 gate,
half the transfer), sign table fp8e4 (signs exact, quarter the transfer).
"""
import numpy as np

N = 21
# f-column split: bank A chunks (via the first qp DMA) and bank B chunks
_WIDTHS_A = (384, 384, 384)
_WIDTHS_B = (299, 299, 298)
ND, NP, NF = 3, 7, 11
CH = 512
NCH = 4

# ----------------------------------------------------------------------------
# host-side small-vector math (O(2^11) work)
# ----------------------------------------------------------------------------


def _ry_v(theta):
    return np.array([np.cos(0.5 * theta), np.sin(0.5 * theta)], dtype=np.complex128)


def _rot_m(phi, theta, omega):
    c, s = np.cos(0.5 * theta), np.sin(0.5 * theta)
    return np.array(
        [
            [np.exp(-0.5j * (phi + omega)) * c, -np.exp(0.5j * (phi - omega)) * s],
            [np.exp(-0.5j * (phi - omega)) * s, np.exp(0.5j * (phi + omega)) * c],
        ],
        dtype=np.complex128,
    )


def _bits(nbits):
    idx = np.arange(1 << nbits)
    return [(idx >> (nbits - 1 - i)) & 1 for i in range(nbits)]


def _chain_vec(vs, prev_bit, nbits):
    bits = _bits(nbits)
    out = np.ones(1 << nbits, np.complex128)
    prev = np.full(1 << nbits, prev_bit)
    for i, v in enumerate(vs):
        out = out * v[bits[i] ^ prev]
        prev = bits[i]
    return out


def _chain_src_idx(nbits, prev_bit):
    bits = _bits(nbits)
    src = np.zeros(1 << nbits, np.int64)
    prev = np.full(1 << nbits, prev_bit)
    for i in range(nbits):
        src = (src << 1) | (bits[i] ^ prev)
        prev = bits[i]
    return src


def _apply_1q(vecs, gate, bit, nbits):
    lead = vecs.shape[:-1]
    a = vecs.reshape(lead + (1 << bit, 2, -1))
    out = np.einsum("ab,...bq->...aq", gate, a)
    return out.reshape(lead + (1 << nbits,))


def build_terms(x, params):
    x = np.asarray(x, np.float64)
    params = np.asarray(params, np.float64)
    v = [np.asarray(_rot_m(*params[0, w]) @ _ry_v(x[w])) for w in range(N)]

    U = np.zeros((2, 8, 128), np.complex128)
    W = np.zeros((2, 2048), np.complex128)
    par_p = np.arange(128) & 1
    for d in range(8):
        c0, c1, c2 = (d >> 2) & 1, (d >> 1) & 1, d & 1
        alpha = v[0][c0] * v[1][c0 ^ c1] * v[2][c1 ^ c2]
        A = _chain_vec([v[w] for w in range(3, 10)], c2, NP)
        U[0, d] = alpha * A * (par_p == 0)
        U[1, d] = alpha * A * (par_p == 1)
    W[0] = _chain_vec([v[w] for w in range(10, 21)], 0, NF)
    W[1] = _chain_vec([v[w] for w in range(10, 21)], 1, NF)

    def apply_layer(U, W, r):
        g = [_rot_m(*params[r, w]) for w in range(N)]
        for w in range(10, 21):
            W = _apply_1q(W, g[w], w - 10, NF)
        for w in range(3, 10):
            U = _apply_1q(U, g[w], w - 3, NP)
        G8 = np.kron(g[0], np.kron(g[1], g[2]))
        U = np.einsum("de,ten->tdn", G8, U)
        return U, W

    U, W = apply_layer(U, W, 1)

    T = U.shape[0]
    Un = np.zeros((2 * T, 8, 128), np.complex128)
    Wn = np.zeros((2 * T, 2048), np.complex128)
    srcf = [_chain_src_idx(NF, s) for s in (0, 1)]
    for d in range(8):
        c0, c1, c2 = (d >> 2) & 1, (d >> 1) & 1, d & 1
        md = (c0 << 2) | ((c0 ^ c1) << 1) | (c1 ^ c2)
        srcp = _chain_src_idx(NP, c2)
        for t in range(T):
            base = U[t, md][srcp]
            for s in (0, 1):
                Un[2 * t + s, d] = base * (par_p == s)
    for t in range(T):
        for s in (0, 1):
            Wn[2 * t + s] = W[t][srcf[s]]
    return apply_layer(Un, Wn, 2)


def sign_tables():
    pbits = np.array(_bits(NP)).T
    fbits = np.array(_bits(NF)).T
    dbits = np.array(_bits(ND)).T
    SA = np.ones((128, N), np.float32)
    SF = np.ones((N, 2048), np.float32)
    SD = np.ones((8, N), np.float32)
    for w in range(N):
        if w <= 2:
            SD[:, w] = (-1.0) ** (dbits[:, : w + 1].sum(1))
        elif w <= 9:
            SD[:, w] = (-1.0) ** (dbits.sum(1))
            SA[:, w] = (-1.0) ** (pbits[:, : w - 2].sum(1))
        else:
            SD[:, w] = (-1.0) ** (dbits.sum(1))
            SA[:, w] = (-1.0) ** (pbits.sum(1))
            SF[w, :] = (-1.0) ** (fbits[:, : w - 9].sum(1))
    return SA, SF, SD


def build_qp(Ud, W):
    """Hermitian rank-16 expansion of |sum_t Ud_t[p] W_t[f]|^2:
    probs = Q^T @ P with Q [16,128], P [16,2048] real."""
    T = Ud.shape[0]
    Q = np.zeros((16, 128), np.float64)
    P = np.zeros((16, 2048), np.float64)
    r = 0
    for t in range(T):
        Q[r] = np.abs(Ud[t]) ** 2
        P[r] = np.abs(W[t]) ** 2
        r += 1
    for t in range(T):
        for u in range(t + 1, T):
            q = Ud[t] * np.conj(Ud[u])
            p = W[t] * np.conj(W[u])
            Q[r] = 2 * q.real
            P[r] = p.real
            Q[r + 6] = 2 * q.imag
            P[r + 6] = -p.imag
            r += 1
    return Q, P


# ----------------------------------------------------------------------------
# device kernel
# ----------------------------------------------------------------------------
_NC_CACHE = {}


def _build_nc():
    from contextlib import ExitStack

    import concourse.bass as bass
    import concourse.mybir as mybir

    f32 = mybir.dt.float32
    f32r = mybir.dt.float32r
    bf16 = mybir.dt.bfloat16
    nc = bass.Bass()
    QPC = N + 2048  # A' cols then P cols
    # 6 f-chunks packed 3-per-PSUM-bank at partition offsets 0/32/64 so
    # each bank's SF contraction is ONE DVE op. Bank A (early-arriving DMA
    # half) is wider than bank B so stt_b hides behind stt_a.
    WA, WB = _WIDTHS_A, _WIDTHS_B
    PWA, PWB = WA[0], WB[0]
    qp_d = nc.declare_dram_parameter("qp", [16, QPC], bf16, isOutput=False)
    sf_d = nc.declare_dram_parameter("sf", [85, PWA + PWB], mybir.dt.float8e4, isOutput=False)
    res_d = nc.declare_dram_parameter("res", [85, 2], f32, isOutput=True)

    with ExitStack() as ctx:
        qp_t = ctx.enter_context(nc.sbuf_tensor("qp_t", [16, QPC], bf16))
        sf_t = ctx.enter_context(nc.sbuf_tensor("sf_t", [85, PWA + PWB], mybir.dt.float8e4))
        scr = ctx.enter_context(nc.sbuf_tensor("scr", [85, PWA], f32))
        res_t = ctx.enter_context(nc.sbuf_tensor("res_t", [85, 2], f32))
        ppA = ctx.enter_context(nc.psum_tensor("ppA", [85, PWA], f32))
        ppB = ctx.enter_context(nc.psum_tensor("ppB", [85, PWB], f32))
        block = ctx.enter_context(nc.Block())
        s_qp = ctx.enter_context(nc.semaphore("s_qp"))
        s_qp2 = ctx.enter_context(nc.semaphore("s_qp2"))
        s_po = ctx.enter_context(nc.semaphore("s_po"))
        s_stt = ctx.enter_context(nc.semaphore("s_stt"))
        s_out = ctx.enter_context(nc.semaphore("s_out"))

        H1 = N + sum(WA)  # A' + bank-A chunks
        @block.sync
        def _(sync):
            sync.dma_start(out=qp_t[:, 0:H1], in_=qp_d[:, 0:H1]).then_inc(s_qp, 16)
            sync.dma_start(out=qp_t[:, H1:QPC], in_=qp_d[:, H1:QPC]).then_inc(
                s_qp2, 16
            )
            sync.wait_ge(s_stt, 2)
            sync.dma_start(out=res_d[:], in_=res_t[:]).then_inc(s_out, 16)

        @block.scalar
        def _(sc):
            # zero the PSUM banks so unwritten lanes contribute exact zeros
            sc.memzero(ppA[:]).then_inc(s_qp, 1)
            sc.memzero(ppB[:]).then_inc(s_qp, 1)

        @block.gpsimd
        def _(g):
            g.dma_start(out=sf_t[:], in_=sf_d[:]).then_inc(s_po, 16)

        @block.tensor
        def _(te):
            lhs = qp_t[:, 0:N]
            te.wait_ge(s_qp, 18)  # qp1 DMA (16) + both memzeros (1+1)
            for half, pp in ((0, ppA), (1, ppB)):
                widths = WA if half == 0 else WB
                if half == 1:
                    te.wait_ge(s_qp2, 16)
                f0 = N + sum(WA) * half
                col = 0
                for q, w in enumerate(widths):
                    mv = qp_t[:, f0 + col : f0 + col + w]
                    te.matmul(
                        pp[32 * q : 32 * q + N, 0:w], lhs, mv, start=True, stop=True
                    ).then_inc(s_po, 1)
                    col += w

        @block.vector
        def _(v):
            for i, pp in ((0, ppA), (1, ppB)):
                pw = PWA if i == 0 else PWB
                # sf DMA (16) + 3/6 matmuls: one fused wait per stt
                v.wait_ge(s_po, 16 + 3 * (i + 1))
                v.scalar_tensor_tensor(
                    out=scr[:, 0:pw],
                    in0=pp[:],
                    scalar=1.0,
                    in1=sf_t[:, PWA * i : PWA * i + pw],
                    op0=mybir.AluOpType.mult,
                    op1=mybir.AluOpType.mult,
                    accum_out=res_t[:, i : i + 1],
                ).then_inc(s_stt, 1)

    return nc


def kernel(x, params):
    import ml_dtypes
    from concourse.bass_utils import run_bass_kernel_spmd

    U, W = build_terms(x, params)
    SA, SF, SD = sign_tables()

    if "nc" not in _NC_CACHE:
        _NC_CACHE["nc"] = _build_nc()
    nc = _NC_CACHE["nc"]

    # sign table packed to the 3-per-bank partition layout
    WA, WB = _WIDTHS_A, _WIDTHS_B
    PWA, PWB = WA[0], WB[0]
    sf_pack = np.zeros((85, PWA + PWB), np.float32)
    off = 0
    for i, widths in enumerate((WA, WB)):
        for q, wd in enumerate(widths):
            sf_pack[32 * q : 32 * q + N, PWA * i : PWA * i + wd] = SF[
                :, off : off + wd
            ]
            off += wd
    sf_bf16 = np.ascontiguousarray(sf_pack.astype(ml_dtypes.float8_e4m3))

    in_maps = []
    for d in range(8):
        Q, P = build_qp(U[:, d, :], W)
        Amat = Q @ SA.astype(np.float64)  # [16, 21]
        qp = np.concatenate([Amat, P], axis=1).astype(ml_dtypes.bfloat16)
        in_maps.append(
            {
                "qp": np.ascontiguousarray(qp),
                "sf": sf_bf16,
            }
        )

    res = run_bass_kernel_spmd(nc, in_maps, core_ids=list(range(8)))
    outs = res.results
    total = np.zeros(N, np.float64)
    for d in range(8):
        r = np.asarray(outs[d]["res"]).astype(np.float64).reshape(85, 2)
        per_wire = sum(r[32 * q : 32 * q + N, :].sum(axis=1) for q in range(3))
        total += SD[d].astype(np.float64) * per_wire
    return total.astype(np.float32)
